# revision 6
# baseline (speedup 1.0000x reference)
"""Trainium2 Bass kernel: EdgeFeatureEncoding scatter-add.

Computes bias[i, j, :] += edge_attr[e] @ W + b over E edges (i, j),
bias shape (N, N, 8) with N = 4096, E = 131072 -> 512 MiB output.

Strategy (8 NeuronCores, SPMD):
- Output rows i are sharded across the 8 cores (512 rows -> 64 MiB each).
- Host-side: bucket edges by destination shard, sort by local flat slot
  d = (i % 512) * N + j.  Groups of edges sharing one (i, j) slot are
  packed into the leading "selection" super-chunk where the device sums
  them with the is_equal/selection-matrix matmul trick (so duplicate
  DMA writes all carry the identical group sum).  All remaining edges
  have unique destinations.
- Device-side: (a) zero-fill the 64 MiB shard with big contiguous DMAs
  from a zeroed SBUF tile, (b) project edges on the PE (per-128-edge
  transpose + matmul with the replicated [128, 8] weight), (c) scatter
  32 B rows into the shard with SWDGE indirect DMA (1024 rows/call).
- Padding edges carry zero features and point at a 128-row trash region
  appended to the table; the trash rows are sliced off on the host.
"""

import os
from dataclasses import dataclass

import numpy as np

H = 8  # n_heads
F = 128  # edge feature dim
CH = 128  # edges per chunk (one partition tile)
SUP_CH = 8  # chunks per super-chunk
SUP_E = CH * SUP_CH  # 1024 edges per super-chunk / indirect DMA
TRASH = 128  # trash rows appended to the shard table
N_CORES = 8


@dataclass(frozen=True)
class _Cfg:
    n_nodes: int
    n_shards: int
    k_edges: int  # padded per-core bucket size (multiple of SUP_E)
    sel_sups: int  # leading super-chunks with duplicate-group handling
    zero_chunks: int  # number of zero-fill DMAs covering the shard

    @property
    def rows(self):
        return self.n_nodes // self.n_shards

    @property
    def table_real(self):
        return self.rows * self.n_nodes  # real rows per shard

    @property
    def table_rows(self):
        return self.table_real + TRASH

    @property
    def nsup(self):
        return self.k_edges // SUP_E


_cache: dict = {}


def _build(cfg: _Cfg):
    import concourse.bacc as bacc
    import concourse.bass as bass
    import concourse.mybir as mybir
    import concourse.tile as tile
    from concourse.masks import make_identity

    f32 = mybir.dt.float32
    i32 = mybir.dt.int32

    nc = bacc.Bacc(
        "TRN2", target_bir_lowering=False, debug=False, num_devices=cfg.n_shards
    )
    xb = nc.dram_tensor("xb", [cfg.k_edges, F], f32, kind="ExternalInput")
    idxb = nc.dram_tensor("idxb", [cfg.nsup, CH, SUP_CH], i32, kind="ExternalInput")
    w = nc.dram_tensor("w", [F, H], f32, kind="ExternalInput")
    brep = nc.dram_tensor("brep", [CH, H], f32, kind="ExternalInput")
    # output table: real shard rows then a small trash region
    table = nc.dram_tensor("table", [cfg.table_rows, H], f32, kind="ExternalOutput")

    zcols = cfg.table_real * H // (cfg.zero_chunks * 128)

    with tile.TileContext(nc) as tc:
        with (
            tc.tile_pool(name="const", bufs=1) as constp,
            tc.tile_pool(name="zero", bufs=1) as zerop,
            tc.tile_pool(name="xin", bufs=3) as xinp,
            tc.tile_pool(name="xt", bufs=4) as xtp,
            tc.tile_pool(name="small", bufs=4) as smallp,
            tc.tile_pool(name="src", bufs=3) as srcp,
            tc.tile_pool(name="psum", bufs=3, space="PSUM") as psp,
        ):
            ident = constp.tile([CH, CH], f32)
            make_identity(nc, ident[:])
            wt = constp.tile([F, H], f32)
            nc.scalar.dma_start(out=wt[:], in_=w.ap())
            bt = constp.tile([CH, H], f32)
            nc.scalar.dma_start(out=bt[:], in_=brep.ap())

            # ---- zero-fill the real shard rows with big contiguous DMAs
            ztile = zerop.tile([128, zcols], f32)
            nc.vector.memset(ztile[:], 0.0)
            zview = table.ap()[: cfg.table_real].rearrange(
                "(c p x) h -> c p (x h)", c=cfg.zero_chunks, p=128
            )
            for c in range(cfg.zero_chunks):
                nc.sync.dma_start(out=zview[c], in_=ztile[:])

            # ---- edge pipeline
            xview = xb.ap().rearrange("(s k p) f -> s p k f", k=SUP_CH, p=CH)
            for s in range(cfg.nsup):
                x_sup = xinp.tile([CH, SUP_CH * F], f32, tag="xin")
                nc.scalar.dma_start(out=x_sup[:], in_=xview[s])
                idx_t = smallp.tile([CH, SUP_CH], i32, tag="idx")
                nc.scalar.dma_start(out=idx_t[:], in_=idxb.ap()[s])
                src_t = srcp.tile([CH, SUP_CH * H], f32, tag="src")
                for k in range(SUP_CH):
                    xt_ps = psp.tile([CH, CH], f32, space="PSUM", tag="xtp")
                    nc.tensor.transpose(
                        out=xt_ps[:],
                        in_=x_sup[:, k * F : (k + 1) * F],
                        identity=ident[:],
                    )
                    xt_sb = xtp.tile([CH, CH], f32, tag="xt")
                    nc.vector.tensor_copy(out=xt_sb[:], in_=xt_ps[:])
                    pj_ps = psp.tile([CH, H], f32, space="PSUM", tag="pj")
                    nc.tensor.matmul(
                        out=pj_ps[:], lhsT=xt_sb[:], rhs=wt[:], start=True, stop=True
                    )
                    if s < cfg.sel_sups:
                        # duplicate-destination handling: rows of a group all
                        # get the group sum via the selection-matrix matmul
                        pj_sb = smallp.tile([CH, H], f32, tag="pjsb")
                        nc.vector.tensor_add(out=pj_sb[:], in0=pj_ps[:], in1=bt[:])
                        idxf = smallp.tile([CH, 1], f32, tag="idxf")
                        nc.vector.tensor_copy(out=idxf[:], in_=idx_t[:, k : k + 1])
                        idt_ps = psp.tile([CH, CH], f32, space="PSUM", tag="xtp")
                        nc.tensor.transpose(
                            out=idt_ps[:],
                            in_=idxf[:].to_broadcast([CH, CH]),
                            identity=ident[:],
                        )
                        idt_sb = xtp.tile([CH, CH], f32, tag="idt")
                        nc.vector.tensor_copy(out=idt_sb[:], in_=idt_ps[:])
                        sel = xtp.tile([CH, CH], f32, tag="sel")
                        nc.vector.tensor_tensor(
                            out=sel[:],
                            in0=idxf[:].to_broadcast([CH, CH]),
                            in1=idt_sb[:],
                            op=mybir.AluOpType.is_equal,
                        )
                        acc_ps = psp.tile([CH, H], f32, space="PSUM", tag="pj")
                        nc.tensor.matmul(
                            out=acc_ps[:], lhsT=sel[:], rhs=pj_sb[:],
                            start=True, stop=True,
                        )
                        nc.vector.tensor_copy(
                            out=src_t[:, k * H : (k + 1) * H], in_=acc_ps[:]
                        )
                    else:
                        nc.vector.tensor_add(
                            out=src_t[:, k * H : (k + 1) * H], in0=pj_ps[:], in1=bt[:]
                        )
                # scatter: HW indirect DMA honors ONE offset per partition
                # (verified on HW), so one call per 128-edge chunk
                for k in range(SUP_CH):
                    nc.gpsimd.indirect_dma_start(
                        out=table.ap(),
                        out_offset=bass.IndirectOffsetOnAxis(
                            ap=idx_t[:, k : k + 1], axis=0
                        ),
                        in_=src_t[:, k * H : (k + 1) * H],
                        in_offset=None,
                    )

    nc.compile()
    return nc


def _prepare(edge_index, edge_attr, n_nodes, n_shards, sel_cap):
    """Bucket edges by shard; sort by dest; pack duplicate groups into the
    leading selection block so no group spans a 128-edge chunk boundary.
    Returns (K, xb list, idx list) with idx premultiplied by H (flat f32
    element offsets)."""
    N = n_nodes
    R = N // n_shards
    table_real = R * N
    i = np.asarray(edge_index[0], dtype=np.int64)
    j = np.asarray(edge_index[1], dtype=np.int64)
    valid = (i >= 0) & (i < N) & (j >= 0) & (j < N)
    eids = np.nonzero(valid)[0]
    i = i[eids]
    j = j[eids]
    shard = i // R
    d = (i - shard * R) * N + j

    edge_attr = np.asarray(edge_attr, dtype=np.float32)

    buckets = []
    max_len = 0
    for s in range(n_shards):
        m = shard == s
        es, ds = eids[m], d[m]
        o = np.argsort(ds, kind="stable")
        es, ds = es[o], ds[o]
        _, start, counts = np.unique(ds, return_index=True, return_counts=True)
        multi = np.nonzero(counts > 1)[0]
        sel_e: list = []
        sel_d: list = []

        def pad_to(n, sel_e=sel_e, sel_d=sel_d):
            while len(sel_e) < n:
                sel_e.append(-1)
                sel_d.append(table_real + (len(sel_d) % TRASH))

        for g in multi:
            st, ln = int(start[g]), int(counts[g])
            assert ln <= CH, f"duplicate group of {ln} edges exceeds chunk"
            used = len(sel_e) % CH
            if ln > CH - used:
                pad_to(len(sel_e) + (CH - used))
            sel_e.extend(es[st : st + ln].tolist())
            sel_d.extend(ds[st : st + ln].tolist())
        assert len(sel_e) <= sel_cap, (
            f"{len(sel_e)} duplicate-group edges exceed selection capacity {sel_cap}"
        )
        pad_to(sel_cap)
        single = np.nonzero(counts == 1)[0]
        be = np.concatenate([np.asarray(sel_e, np.int64), es[start[single]]])
        bd = np.concatenate([np.asarray(sel_d, np.int64), ds[start[single]]])
        buckets.append((be, bd))
        max_len = max(max_len, len(be))

    K = -(-max_len // SUP_E) * SUP_E
    xs, ids = [], []
    for be, bd in buckets:
        n = len(be)
        q = np.arange(K - n)
        be = np.concatenate([be, np.full(K - n, -1, np.int64)])
        bd = np.concatenate([bd, table_real + (q % TRASH)])
        x = np.zeros((K, F), np.float32)
        real = be >= 0
        x[real] = edge_attr[be[real]]
        idx = bd.astype(np.int32)  # table row indices
        idx = idx.reshape(K // SUP_E, SUP_CH, CH).transpose(0, 2, 1)
        xs.append(x)
        ids.append(np.ascontiguousarray(idx))
    return K, xs, ids


LAST_EXEC_NS = None
LAST_RESULTS = None


def kernel(edge_index, edge_attr, num_nodes, W, b):
    from concourse.bass_utils import run_bass_kernel_spmd

    global LAST_EXEC_NS, LAST_RESULTS
    N = int(num_nodes)
    S = N_CORES
    R = N // S
    table_real = R * N

    sel_sups = 1
    K, xs, ids = _prepare(edge_index, edge_attr, N, S, sel_cap=sel_sups * SUP_E)
    cfg = _Cfg(
        n_nodes=N, n_shards=S, k_edges=K, sel_sups=sel_sups,
        zero_chunks=16,
    )
    nc = _cache.get(cfg)
    if nc is None:
        nc = _build(cfg)
        _cache[cfg] = nc

    W_np = np.ascontiguousarray(np.asarray(W, dtype=np.float32))
    b_rep = np.ascontiguousarray(
        np.broadcast_to(np.asarray(b, dtype=np.float32), (CH, H))
    )
    in_maps = [
        {"xb": xs[s], "idxb": ids[s], "w": W_np, "brep": b_rep} for s in range(S)
    ]
    trace = bool(int(os.environ.get("EDGE_KERNEL_TRACE", "0")))
    res = run_bass_kernel_spmd(
        nc, in_maps, core_ids=list(range(S)), trace=trace
    )
    LAST_EXEC_NS = res.exec_time_ns
    LAST_RESULTS = res
    out = np.concatenate(
        [r["table"][:table_real].reshape(R, N, H) for r in res.results],
        axis=0,
    )
    return out
